# revision 38
# baseline (speedup 1.0000x reference)
"""MultiHeadLatentAttention on 8 Trainium2 NeuronCores.

Sharding: 2 batches x 4 head-groups (4 heads each) = 8 cores.
Each core computes, for its batch b and heads [4*hg, 4*hg+4):
  q = x[b] @ Wq[:, cols]                  (computed transposed: qT [512, T])
  latent_new = LN(x[b] @ Wdown)           (computed transposed, replicated on
                                           the 4 cores of the same batch)
  kT = (latent @ Wk[:, cols]).T           v = latent @ Wv[:, cols]
  scores.T, softmax (no max-subtraction; |scores| <= ~3), PV accumulation
  o_partial = attn_out @ Wo[rows, :]      -> [T, D] partial sum (fp16)
Host sums the 4 partials per batch (f32) and stacks the 2 batches.

Dtype strategy: fp16 everywhere for 2-byte tensors (same PE rate as
bf16/fp32r -- 1 col/cycle -- but 8x lower quantization error than bf16,
and it halves HBM traffic, which un-bottlenecks phase A's x/weight
streaming). All accumulation is f32 in PSUM; LN statistics in f32.

Schedule notes (from trace analysis of the bf16 baseline):
- PE sustained rate is ~263 ns per 512-col matmul regardless of dtype;
  every gap also costs a pstate re-ramp, so the layout aims to keep the
  PE stream dependency-free: deep DMA prefetch, psum double-buffering.
- Phase C was ACT-paced (416 EXPs x ~582ns ~= PE work).  Scores for two
  adjacent key blocks now land in one [128,1024] psum pair (2 banks) and
  are EXP'd by a single ACT instruction, cutting ACT per-tile overhead.
- Head: the first matmul only needs wd chunk 0 + x quarter tile 0, so
  those two DMAs are issued before all other constants.
- The softmax denominator accumulates exp'd pairs 1024-wide on DVE, is
  folded to 512 and partition-reduced by a ones-matmul per 8 key blocks.
- Output is written fp16 (halves the o DMA; host sums partials in f32).
"""

import numpy as np

N_HEADS = 16
T = 2048
D = 2048
LAT = 512
PAST = 2048
S = PAST + T  # 4096, below the 8192 cache cap
HD = D // N_HEADS  # 128
HPC = 4  # heads per core
LN_EPS = 1e-5
SCALE = 1.0 / float(np.sqrt(HD))
NJB = S // 128  # 32 key blocks
NTT = T // 512  # 4 query tiles
NDC = D // 128  # 16
NLC = LAT // 128  # 4

_CACHE = {}


def _r(ap):
    import concourse.mybir as mybir

    return ap.bitcast(mybir.dt.float32r)


def _build():
    import concourse.bacc as bacc
    import concourse.mybir as mybir
    import concourse.tile as tile

    f32 = mybir.dt.float32
    f32r = mybir.dt.float32r
    f16 = mybir.dt.float16
    AF = mybir.ActivationFunctionType
    OP = mybir.AluOpType

    nc = bacc.Bacc("TRN2", target_bir_lowering=False, debug=False, num_devices=8)

    # xq/wq/wd/lpT arrive host-swizzled to partition-major ([128, ...]
    # flattened) so each loads with one full-rate DMA: their natural row
    # length (512 fp16 = 1KB) would otherwise halve DMA throughput and
    # the per-chunk issues would clog the queues.
    xT = nc.dram_tensor("xT", [D, T], f16, kind="ExternalInput")
    xq = nc.dram_tensor("xq", [128, NDC * 512], f16, kind="ExternalInput")
    lpT = nc.dram_tensor("lpT", [128, NLC * PAST], f16, kind="ExternalInput")
    wq = nc.dram_tensor("wq", [128, NDC * LAT], f16, kind="ExternalInput")
    wd = nc.dram_tensor("wd", [128, NDC * LAT], f16, kind="ExternalInput")
    wk = nc.dram_tensor("wk", [LAT, LAT], f16, kind="ExternalInput")
    wv = nc.dram_tensor("wv", [LAT, LAT], f16, kind="ExternalInput")
    wo = nc.dram_tensor("wo", [LAT, D], f16, kind="ExternalInput")
    g = nc.dram_tensor("g", [LAT], f32, kind="ExternalInput")
    b = nc.dram_tensor("bb", [LAT], f32, kind="ExternalInput")
    mask = nc.dram_tensor("mask", [128, 896], f16, kind="ExternalInput")
    o = nc.dram_tensor("o", [T, D], f16, kind="ExternalOutput")

    with tile.TileContext(nc) as tc:
        with (
            tc.tile_pool(name="consts", bufs=1) as consts,
            tc.tile_pool(name="persist", bufs=1) as persist,
            tc.tile_pool(name="lpp", bufs=1) as lpp,
            tc.tile_pool(name="ccd", bufs=1, space="DRAM") as ccd,
        ):
            # mask[jj, c] = 1 iff c >= jj + 384, so cols [512, 640) are all
            # ones for every partition — doubles as the ones matrix for the
            # softmax-denominator matmul.
            mask_sb = consts.tile([128, 896], f16)
            g_sb = consts.tile([128, NLC], f32)
            b_sb = consts.tile([128, NLC], f32)
            eps_sb = consts.tile([128, 1], f32)
            nc.vector.memset(eps_sb, LN_EPS)
            ones_f32 = consts.tile([128, 128], f32)
            nc.vector.memset(ones_f32, 1.0)
            ones_sb = mask_sb[:, 512:640]

            qT_sb = persist.tile([128, HPC, T], f16)  # q transposed, per head
            latn_sb = persist.tile([128, NLC, T], f16)  # new latent (gathered)
            wk_sb = persist.tile([128, NLC, LAT], f16)
            wv_sb = persist.tile([128, NLC, LAT], f16)
            lp_sb = lpp.tile([128, NLC, PAST], f16)  # past latent
            # k/v weights ride the gpsimd DMA queue, in parallel with the
            # sync-queue weight/x streams of phase A
            nc.gpsimd.dma_start(
                wk_sb[:], wk[:, :].rearrange("(lc p) n -> p lc n", p=128)
            )
            nc.gpsimd.dma_start(
                wv_sb[:], wv[:, :].rearrange("(lc p) n -> p lc n", p=128)
            )
            cc_in = ccd.tile([LAT, 512], f16)
            cc_out = ccd.tile([4 * LAT, 512], f16)

            # ---- Phase A: latent down-projection + LayerNorm (first, so the
            # LN tail overlaps the q matmuls), then q projection.
            with (
                tc.tile_pool(name="wA", bufs=1) as wA,
                tc.tile_pool(name="stats", bufs=1) as stats,
                tc.tile_pool(name="psA", bufs=1, space="PSUM") as psA,
            ):
                wq_sb = wA.tile([128, NDC, LAT], f16)
                wd_sb = wA.tile([128, NDC, LAT], f16)
                # Weights ride the scalar (ACT) DMA queue, x the sync queue.

                latqb_sb = wA.tile([128, NLC, 512], f16)

                def ln_emit_stats():
                    # LN stats via ones-matmuls into PSUM (the qp2 bank pair)
                    # — keeps gpsimd out of phase A so the pool-close barrier
                    # cannot queue behind the collective on the gpsimd queue.
                    spair = psA.tile([128, 1024], f32, tag="qp3", name="sstat")
                    ssum = spair[:, 0:512]
                    ssq = spair[:, 512:1024]
                    for lc in range(NLC):
                        nc.tensor.matmul(
                            ssum,
                            lhsT=_r(ones_f32),
                            rhs=latq_sb[:, lc, :],
                            start=(lc == 0),
                            stop=(lc == NLC - 1),
                        )
                    for lc in range(NLC):
                        nc.tensor.matmul(
                            ssq,
                            lhsT=_r(ones_f32),
                            rhs=sqs[lc],
                            start=(lc == 0),
                            stop=(lc == NLC - 1),
                        )
                    mu = stats.tile([128, 512], f32, tag="mu", name="mu")
                    nc.vector.tensor_scalar_mul(mu, ssum, 1.0 / LAT)
                    vtmp = stats.tile([128, 512], f32, tag="vtmp", name="vtmp")
                    nc.vector.tensor_mul(vtmp, mu, mu)
                    sd = stats.tile([128, 512], f32, tag="sd", name="sd")
                    nc.vector.scalar_tensor_tensor(
                        out=sd,
                        in0=ssq,
                        scalar=1.0 / LAT,
                        in1=vtmp,
                        op0=OP.mult,
                        op1=OP.subtract,
                    )
                    # normalize tail, staged into the q-pass stream
                    box = {"mu": mu, "sd": sd}
                    ops = []
                    ops.append(lambda: nc.scalar.activation(
                        box["sd"], box["sd"], AF.Sqrt, bias=eps_sb))

                    def mkrstd():
                        box["rstd"] = stats.tile(
                            [128, 512], f32, tag="rstd", name="rstd")
                        nc.vector.reciprocal_approx_fast(box["rstd"], box["sd"])

                    ops.append(mkrstd)
                    for lc in range(NLC):
                        def mk1(lc=lc):
                            box["t1"] = stats.tile(
                                [128, 512], f32, tag="sq0", name="t1")
                            nc.vector.tensor_sub(
                                box["t1"], latq_sb[:, lc, :], box["mu"])

                        def mk2():
                            box["t2"] = stats.tile(
                                [128, 512], f32, tag="sq1", name="t2")
                            nc.vector.tensor_mul(
                                box["t2"], box["t1"], box["rstd"])

                        ops.append(mk1)
                        ops.append(mk2)
                        ops.append(lambda lc=lc: nc.vector.tensor_scalar(
                            latqb_sb[:, lc, :], box["t2"], g_sb[:, lc : lc + 1],
                            b_sb[:, lc : lc + 1], OP.mult, OP.add))
                    return ops

                pend_ln = []

                def drain_ln(k):
                    for _ in range(min(k, len(pend_ln))):
                        pend_ln.pop(0)()

                # down-projection for this core's T/4 token quarter
                # (64 matmuls).  x and its quarter are streamed once and
                # stay RESIDENT in SBUF (10MB fp16): the q projection then
                # has zero DMA dependency, so neither the x stream nor the
                # AllGather's DRAM round-trip can stall the PE.  PSUM is
                # managed as four [128,1024] bank pairs so psum->SBUF
                # copies run 1024 wide.
                latq_sb = wA.tile([128, NLC, 512], f32r)
                x_sb = wA.tile([128, NDC, T], f16)
                xq_sb = wA.tile([128, NDC, 512], f16)
                dpair = [
                    psA.tile([128, 1024], f32, tag=f"qp{i}", name=f"d_ps{i}")
                    for i in range(2)
                ]
                d_ps = [dpair[lc // 2][:, (lc % 2) * 512 : (lc % 2 + 1) * 512]
                        for lc in range(NLC)]
                # sync queue: x-quarter halves first (down-critical; each a
                # single full-rate 1MB DMA), consts, then the full x rows
                # in groups of 4 rows (4KB descriptors, full rate)
                xq_src = xq[:, :].rearrange("p (c t) -> p c t", c=NDC)
                nc.sync.dma_start(out=xq_sb[:, 0:8, :], in_=xq_src[:, 0:8, :])
                nc.sync.dma_start(out=xq_sb[:, 8:16, :], in_=xq_src[:, 8:16, :])
                # x rows in 1MB pairs right behind the quarter: fine
                # completion granularity so tp0's early chunks never wait on
                # a fat group, cheap issues.  Constants after (mask isn't
                # read until phase C).
                for lo in range(0, NDC, 2):
                    nc.sync.dma_start(
                        out=x_sb[:, lo : lo + 2, :],
                        in_=xT[lo * 128 : (lo + 2) * 128, :].rearrange(
                            "(c p) n -> p c n", p=128
                        ),
                    )
                nc.sync.dma_start(out=mask_sb, in_=mask[:, :])
                nc.sync.dma_start(
                    out=g_sb, in_=g[:].rearrange("(lc p) -> p lc", p=128)
                )
                nc.sync.dma_start(
                    out=b_sb, in_=b[:].rearrange("(lc p) -> p lc", p=128)
                )
                # scalar queue: wd halves then wq halves — partition-major
                # single full-rate DMAs.  (The past-latent load is deferred
                # past the critical window; see below.)
                wd_src = wd[:, :].rearrange("p (c n) -> p c n", c=NDC)
                wq_src = wq[:, :].rearrange("p (c n) -> p c n", c=NDC)
                nc.scalar.dma_start(out=wd_sb[:, 0:8, :], in_=wd_src[:, 0:8, :])
                nc.scalar.dma_start(out=wd_sb[:, 8:16, :], in_=wd_src[:, 8:16, :])
                nc.scalar.dma_start(out=wq_sb[:, 0:8, :], in_=wq_src[:, 0:8, :])
                nc.scalar.dma_start(out=wq_sb[:, 8:16, :], in_=wq_src[:, 8:16, :])
                for dc in range(NDC):
                    for lc in range(NLC):
                        nc.tensor.matmul(
                            d_ps[lc],
                            lhsT=wd_sb[:, dc, lc * 128 : (lc + 1) * 128],
                            rhs=xq_sb[:, dc, :],
                            start=(dc == 0),
                            stop=(dc == NDC - 1),
                        )
                for pi in range(2):
                    dst = latq_sb[:, 2 * pi : 2 * pi + 2, :]
                    src = dpair[pi][:].rearrange("p (a b) -> p a b", a=2)
                    if pi == 0:
                        nc.vector.tensor_copy(dst, src)
                    else:
                        nc.scalar.copy(out=dst, in_=src)
                # squares, LN stats and the normalize chain immediately:
                # latqb must be in DRAM early so the AllGather can trigger
                # at tp0-end (with x resident there is no DMA stream left
                # for its D2D traffic to disturb)
                sqs = []
                for lc in range(NLC):
                    sq = stats.tile([128, 512], f32r, tag=f"sq{lc}", name="sq")
                    nc.scalar.square(sq, latq_sb[:, lc, :])
                    sqs.append(sq)
                pend_ln.extend(ln_emit_stats())

                # q projection from resident x.  Pair pi = half*2 + qc//2.
                def q_mm(pr, pi, dc, tp):
                    half, qg = pi // 2, pi % 2
                    for sub in range(2):
                        qc = qg * 2 + sub
                        nc.tensor.matmul(
                            pr[:, sub * 512 : (sub + 1) * 512],
                            lhsT=wq_sb[:, dc, qc * 128 : (qc + 1) * 128],
                            rhs=x_sb[
                                :, dc,
                                tp * 1024 + half * 512 : tp * 1024
                                + (half + 1) * 512,
                            ],
                            start=(dc == 0),
                            stop=(dc == NDC - 1),
                        )

                def q_copy(pr, pi, tp, eng):
                    half, qg = pi // 2, pi % 2
                    tt = tp * 2 + half
                    dst = qT_sb[:, qg * 2 : qg * 2 + 2, tt * 512 : (tt + 1) * 512]
                    src = pr[:].rearrange("p (a b) -> p a b", a=2)
                    if eng == 0:
                        nc.scalar.copy(out=dst, in_=src)
                    else:
                        nc.vector.tensor_copy(dst, src)

                # tp0: dc-major, qp2 first within each dc (qp0/qp1 wait for
                # the latq copies, qp3 for the stats readers — both clear
                # within the first few matmuls).  The LN normalize chain
                # drains into this stream.
                q0 = {
                    pi: psA.tile([128, 1024], f32, tag=f"qp{pi}", name=f"q0_{pi}")
                    for pi in (2, 0, 1, 3)
                }
                for dc in range(NDC):
                    drain_ln(4)
                    for pi in (2, 0, 1, 3):
                        q_mm(q0[pi], pi, dc, 0)
                    if dc == 2:
                        # past-latent first half, deferred out of the
                        # down/tp0-critical DMA window (phase B needs it
                        # ~60us from now)
                        lp_src = lpT[:, :].rearrange(
                            "p (lc t) -> p lc t", lc=NLC
                        )
                        nc.scalar.dma_start(
                            out=lp_sb[:, :, 0:1024], in_=lp_src[:, :, 0:1024]
                        )
                for k, pi in enumerate((2, 3, 0, 1)):
                    q_copy(q0[pi], pi, 0, k % 2)
                drain_ln(999)
                # ship this core's normalized latent quarter and fire the
                # gather NOW: its DRAM round-trip overlaps tp1 (which reads
                # only resident SBUF) and B-past, and latn lands with wide
                # margin before B-new
                nc.sync.dma_start(
                    out=cc_in[:].rearrange("(lc p) t -> p lc t", p=128),
                    in_=latqb_sb[:],
                )
                nc.gpsimd.collective_compute(
                    "AllGather",
                    mybir.AluOpType.bypass,
                    replica_groups=[[0, 1, 2, 3], [4, 5, 6, 7]],
                    ins=[cc_in.opt()],
                    outs=[cc_out.opt()],
                )
                for rk in range(4):
                    nc.gpsimd.dma_start(
                        latn_sb[:, :, rk * 512 : (rk + 1) * 512],
                        cc_out[rk * 512 : (rk + 1) * 512, :].rearrange(
                            "(lc p) t -> p lc t", p=128
                        ),
                    )

                # tp1: bank-pair-major from resident x — pure PE streaming
                def q_block(pi):
                    pr = psA.tile(
                        [128, 1024], f32, tag=f"qp{pi}", name=f"q1_{pi}"
                    )
                    for dc in range(NDC):
                        q_mm(pr, pi, dc, 1)
                    return pr

                q1 = {}
                for pi in (2, 0, 1, 3):
                    q1[pi] = q_block(pi)
                for k, pi in enumerate((0, 1, 2, 3)):
                    q_copy(q1[pi], pi, 1, k % 2)

            # ---- Phase B: k/v up-projection for all 4 heads
            with tc.tile_pool(name="kvbuf", bufs=1) as kvp:
                kT_sb = kvp.tile([128, HPC, S], f16)
                v_sb = kvp.tile([128, NJB, LAT], f16)
                ao_sb = kvp.tile([128, HPC, T], f16)  # attn out transposed
                with tc.tile_pool(name="psB", bufs=1, space="PSUM") as psB:

                    def latf(lc, g_):
                        if g_ < PAST // 512:
                            return lp_sb[:, lc, g_ * 512 : (g_ + 1) * 512]
                        gg = g_ - PAST // 512
                        return latn_sb[:, lc, gg * 512 : (gg + 1) * 512]

                    for g_ in range(S // 512):
                        ssl = slice(g_ * 512, (g_ + 1) * 512)
                        if g_ == 0:
                            # prefetch the second past-latent half (scalar
                            # queue; gpsimd is owned by the collective)
                            lps = lpT[:, :].rearrange(
                                "p (lc t) -> p lc t", lc=NLC
                            )
                            nc.scalar.dma_start(
                                out=lp_sb[:, :, 1024:2048],
                                in_=lps[:, :, 1024:2048],
                            )
                        if g_ == 5:
                            # pre-load the ACT Exp table (1.28us) here, where
                            # the scalar engine has slack, instead of at the
                            # first attention EXP
                            warm = consts.tile([128, 1], f16, name="warm")
                            nc.scalar.activation(warm, eps_sb, AF.Exp)
                        for h in range(HPC):
                            hsl = slice(h * 128, (h + 1) * 128)
                            k_ps = psB.tile(
                                [128, 512], f32, tag="kps", bufs=3, name="k_ps"
                            )
                            for lc in range(NLC):
                                nc.tensor.matmul(
                                    k_ps,
                                    lhsT=wk_sb[:, lc, hsl],
                                    rhs=latf(lc, g_),
                                    start=(lc == 0),
                                    stop=(lc == NLC - 1),
                                )
                            if h % 2 == 0:
                                nc.vector.tensor_copy(kT_sb[:, h, ssl], k_ps)
                            else:
                                nc.scalar.copy(out=kT_sb[:, h, ssl], in_=k_ps)
                        for j4 in range(4):
                            v_ps = psB.tile(
                                [128, 512], f32, tag="vps", bufs=3, name="v_ps"
                            )
                            for lc in range(NLC):
                                nc.tensor.matmul(
                                    v_ps,
                                    lhsT=latf(lc, g_)[
                                        :, j4 * 128 : (j4 + 1) * 128
                                    ],
                                    rhs=wv_sb[:, lc, :],
                                    start=(lc == 0),
                                    stop=(lc == NLC - 1),
                                )
                            if j4 % 2 == 0:
                                nc.scalar.copy(out=v_sb[:, g_ * 4 + j4, :], in_=v_ps)
                            else:
                                nc.vector.tensor_copy(v_sb[:, g_ * 4 + j4, :], v_ps)

                # ---- Phase C: attention per head / query tile.  Scores for
                # two adjacent key blocks share one [128,1024] psum pair and
                # one EXP instruction.
                with (
                    tc.tile_pool(name="pp", bufs=8) as pp,
                    tc.tile_pool(name="pdp", bufs=2) as pdp,
                    tc.tile_pool(name="ctmp", bufs=2) as ctmp,
                    tc.tile_pool(name="psC", bufs=1, space="PSUM") as psC,
                ):
                  wo_sb = kvp.tile([128, HPC, D], f16)
                  # wo rides the gpsimd queue behind the latn scatter: it
                  # lands mid-B, off the sync queue's congested A->B window
                  nc.gpsimd.dma_start(
                      wo_sb[:],
                      wo[:, :].rearrange("(hc p) n -> p hc n", p=128),
                  )
                  for h in range(HPC):
                    hsl = slice(h * 128, (h + 1) * 128)
                    for tt in range(NTT):
                        tsl = slice(tt * 512, (tt + 1) * 512)
                        nvis = PAST // 128 + 4 * (tt + 1)  # 20/24/28/32
                        npair = nvis // 2
                        # attention accumulator and softmax denominator share
                        # one psum tile (same lifetime)
                        ad = psC.tile([128, 1024], f32, tag="ad", bufs=2, name="ad")
                        attn_ps = ad[:, 0:512]
                        den_ps = ad[:, 512:1024]
                        grp = []
                        e = None
                        den_started = False
                        for jp in range(npair):
                            jb0, jb1 = 2 * jp, 2 * jp + 1
                            sp = psC.tile(
                                [128, 1024], f32, tag="sp", bufs=2, name="s_ps"
                            )
                            nc.tensor.matmul(
                                sp[:, 0:512],
                                lhsT=kT_sb[:, h, jb0 * 128 : (jb0 + 1) * 128],
                                rhs=qT_sb[:, h, tsl],
                                start=True,
                                stop=True,
                            )
                            nc.tensor.matmul(
                                sp[:, 512:1024],
                                lhsT=kT_sb[:, h, jb1 * 128 : (jb1 + 1) * 128],
                                rhs=qT_sb[:, h, tsl],
                                start=True,
                                stop=True,
                            )
                            pr = pp.tile([128, 1024], f16, tag="p", name="p")
                            nc.scalar.activation(pr, sp, AF.Exp, scale=SCALE)
                            for half, jb in ((0, jb0), (1, jb1)):
                                jbn = jb - PAST // 128
                                if jbn >= 0 and jbn // 4 == tt:
                                    rr = (jbn % 4) * 128
                                    hslc = slice(half * 512, (half + 1) * 512)
                                    nc.vector.tensor_mul(
                                        pr[:, hslc],
                                        pr[:, hslc],
                                        mask_sb[:, 384 - rr : 896 - rr],
                                    )
                            for half, jb in ((0, jb0), (1, jb1)):
                                # diagonal blocks: only queries >= rr are
                                # visible (the mask zeroed the rest), so the
                                # PV matmul streams just the visible columns
                                jbn = jb - PAST // 128
                                rr = 0
                                if jbn >= 0 and jbn // 4 == tt:
                                    rr = (jbn % 4) * 128
                                nc.tensor.matmul(
                                    attn_ps[:, rr:512],
                                    lhsT=v_sb[:, jb, hsl],
                                    rhs=pr[:, half * 512 + rr : (half + 1) * 512],
                                    start=(jp == 0 and half == 0),
                                    stop=(jp == npair - 1 and half == 1),
                                    skip_group_check=True,
                                )
                            # denominator: accumulate exp'd pairs 1024-wide
                            # on DVE, fold + ones-matmul per <=8 pairs
                            grp.append(pr)
                            if len(grp) == 2:
                                e = pdp.tile(
                                    [128, 1024], f16, tag="pd", bufs=2, name="e"
                                )
                                nc.vector.tensor_add(e, grp[0], grp[1])
                            elif len(grp) > 2:
                                nc.vector.tensor_add(e, e, pr)
                            if len(grp) == 8 or jp == npair - 1:
                                ef = pdp.tile(
                                    [128, 512], f16, tag="pf", bufs=2, name="ef"
                                )
                                nc.vector.tensor_add(
                                    ef, e[:, 0:512], e[:, 512:1024]
                                )
                                nc.tensor.matmul(
                                    den_ps,
                                    lhsT=ones_sb,
                                    rhs=ef,
                                    start=not den_started,
                                    stop=(jp == npair - 1),
                                )
                                den_started = True
                                grp = []
                        rec = ctmp.tile([128, 512], f32, tag="rec", name="rec")
                        nc.vector.reciprocal_approx_fast(rec, den_ps)
                        nc.vector.tensor_mul(ao_sb[:, h, tsl], attn_ps, rec)

                # ---- Phase D: output projection (fp16), 1024-wide copies
                # and DMA descriptors
                with (
                    tc.tile_pool(name="ost", bufs=4) as ost,
                    tc.tile_pool(name="psD", bufs=1, space="PSUM") as psD,
                ):
                    for dt2 in range(D // 1024):
                        for tc_ in range(T // 128):
                            o_ps = psD.tile(
                                [128, 1024], f32, tag="ops", bufs=4, name="o_ps"
                            )
                            for half in range(2):
                                csl = slice(
                                    (dt2 * 2 + half) * 512,
                                    (dt2 * 2 + half + 1) * 512,
                                )
                                for hc in range(HPC):
                                    nc.tensor.matmul(
                                        o_ps[:, half * 512 : (half + 1) * 512],
                                        lhsT=ao_sb[
                                            :, hc, tc_ * 128 : (tc_ + 1) * 128
                                        ],
                                        rhs=wo_sb[:, hc, csl],
                                        start=(hc == 0),
                                        stop=(hc == HPC - 1),
                                    )
                            o_sb = ost.tile(
                                [128, 1024], f16, tag="osb", name="o_sb"
                            )
                            if tc_ % 2 == 0:
                                nc.scalar.copy(out=o_sb, in_=o_ps)
                            else:
                                nc.vector.tensor_copy(o_sb, o_ps)
                            nc.sync.dma_start(
                                out=o[
                                    tc_ * 128 : (tc_ + 1) * 128,
                                    dt2 * 1024 : (dt2 + 1) * 1024,
                                ],
                                in_=o_sb,
                            )

    nc.compile()
    return nc


def _get_nc():
    if "nc" not in _CACHE:
        _CACHE["nc"] = _build()
    return _CACHE["nc"]


def _make_mask():
    # B[jj, c] = 1.0 iff c >= jj + 384; sliced at 384-r it gives the
    # causal staircase "visible iff i >= jj + r" for r in {0,128,256,384}.
    jj = np.arange(128)[:, None]
    cc = np.arange(896)[None, :]
    return (cc >= jj + 384)


def _swz(a):
    # [C*128, N] -> partition-major [128, C*N]: row p holds chunks
    # (0..C-1) of the original rows {c*128+p}, each N wide and contiguous
    c = a.shape[0] // 128
    return np.ascontiguousarray(
        a.reshape(c, 128, a.shape[1]).transpose(1, 0, 2).reshape(128, -1)
    )


def _in_maps(x, latent_prev, Wq, Wdown, Wk_up, Wv_up, ln_g, ln_b, Wo):
    f = lambda a: np.ascontiguousarray(np.asarray(a, dtype=np.float32))
    h = lambda a: np.ascontiguousarray(
        np.asarray(a, dtype=np.float32).astype(np.float16)
    )
    mask = _make_mask().astype(np.float16)
    wd_h = _swz(h(Wdown))
    maps = []
    for bi in range(2):
        xTb = h(np.asarray(x)[bi].T)
        lpTb = _swz(h(np.asarray(latent_prev)[bi].T))
        for hg in range(4):
            sl = slice(hg * 512, (hg + 1) * 512)
            maps.append(
                {
                    "xT": xTb,
                    "xq": _swz(np.ascontiguousarray(xTb[:, sl])),
                    "lpT": lpTb,
                    "wq": _swz(h(np.asarray(Wq)[:, sl])),
                    "wd": wd_h,
                    "wk": h(np.asarray(Wk_up)[:, sl]),
                    "wv": h(np.asarray(Wv_up)[:, sl]),
                    "wo": h(np.asarray(Wo)[sl, :]),
                    "g": f(ln_g),
                    "bb": f(ln_b),
                    "mask": mask,
                }
            )
    return maps


def run(trace=False, **inputs):
    from concourse.bass_utils import run_bass_kernel_spmd

    nc = _get_nc()
    maps = _in_maps(**inputs)
    res = run_bass_kernel_spmd(nc, maps, core_ids=list(range(8)), trace=trace)
    outs = [
        np.asarray(res.results[c]["o"], dtype=np.float32) for c in range(8)
    ]
    out = np.stack(
        [
            outs[0] + outs[1] + outs[2] + outs[3],
            outs[4] + outs[5] + outs[6] + outs[7],
        ],
        axis=0,
    ).astype(np.float32)
    return out, res


def kernel(**inputs):
    out, _ = run(trace=False, **inputs)
    return out


# revision 39
# speedup vs baseline: 1.0649x; 1.0649x over previous
"""MultiHeadLatentAttention on 8 Trainium2 NeuronCores.

Sharding: 2 batches x 4 head-groups (4 heads each) = 8 cores.
Each core computes, for its batch b and heads [4*hg, 4*hg+4):
  q = x[b] @ Wq[:, cols]                  (computed transposed: qT [512, T])
  latent_new = LN(x[b] @ Wdown)           (computed transposed, replicated on
                                           the 4 cores of the same batch)
  kT = (latent @ Wk[:, cols]).T           v = latent @ Wv[:, cols]
  scores.T, softmax (no max-subtraction; |scores| <= ~3), PV accumulation
  o_partial = attn_out @ Wo[rows, :]      -> [T, D] partial sum (fp16)
Host sums the 4 partials per batch (f32) and stacks the 2 batches.

Dtype strategy: fp16 everywhere for 2-byte tensors (same PE rate as
bf16/fp32r -- 1 col/cycle -- but 8x lower quantization error than bf16,
and it halves HBM traffic, which un-bottlenecks phase A's x/weight
streaming). All accumulation is f32 in PSUM; LN statistics in f32.

Schedule notes (from trace analysis of the bf16 baseline):
- PE sustained rate is ~263 ns per 512-col matmul regardless of dtype;
  every gap also costs a pstate re-ramp, so the layout aims to keep the
  PE stream dependency-free: deep DMA prefetch, psum double-buffering.
- Phase C was ACT-paced (416 EXPs x ~582ns ~= PE work).  Scores for two
  adjacent key blocks now land in one [128,1024] psum pair (2 banks) and
  are EXP'd by a single ACT instruction, cutting ACT per-tile overhead.
- Head: the first matmul only needs wd chunk 0 + x quarter tile 0, so
  those two DMAs are issued before all other constants.
- The softmax denominator accumulates exp'd pairs 1024-wide on DVE, is
  folded to 512 and partition-reduced by a ones-matmul per 8 key blocks.
- Output is written fp16 (halves the o DMA; host sums partials in f32).
"""

import numpy as np

N_HEADS = 16
T = 2048
D = 2048
LAT = 512
PAST = 2048
S = PAST + T  # 4096, below the 8192 cache cap
HD = D // N_HEADS  # 128
HPC = 4  # heads per core
LN_EPS = 1e-5
SCALE = 1.0 / float(np.sqrt(HD))
NJB = S // 128  # 32 key blocks
NTT = T // 512  # 4 query tiles
NDC = D // 128  # 16
NLC = LAT // 128  # 4

_CACHE = {}


def _r(ap):
    import concourse.mybir as mybir

    return ap.bitcast(mybir.dt.float32r)


def _build():
    import concourse.bacc as bacc
    import concourse.mybir as mybir
    import concourse.tile as tile

    f32 = mybir.dt.float32
    f32r = mybir.dt.float32r
    f16 = mybir.dt.float16
    AF = mybir.ActivationFunctionType
    OP = mybir.AluOpType

    nc = bacc.Bacc("TRN2", target_bir_lowering=False, debug=False, num_devices=8)

    # xq/wq/wd/lpT arrive host-swizzled to partition-major ([128, ...]
    # flattened) so each loads with one full-rate DMA: their natural row
    # length (512 fp16 = 1KB) would otherwise halve DMA throughput and
    # the per-chunk issues would clog the queues.
    xT = nc.dram_tensor("xT", [D, T], f16, kind="ExternalInput")
    xq = nc.dram_tensor("xq", [128, NDC * 512], f16, kind="ExternalInput")
    lpT = nc.dram_tensor("lpT", [128, NLC * PAST], f16, kind="ExternalInput")
    wq = nc.dram_tensor("wq", [128, NDC * LAT], f16, kind="ExternalInput")
    wd = nc.dram_tensor("wd", [128, NDC * LAT], f16, kind="ExternalInput")
    wk = nc.dram_tensor("wk", [LAT, LAT], f16, kind="ExternalInput")
    wv = nc.dram_tensor("wv", [LAT, LAT], f16, kind="ExternalInput")
    wo = nc.dram_tensor("wo", [LAT, D], f16, kind="ExternalInput")
    g = nc.dram_tensor("g", [LAT], f32, kind="ExternalInput")
    b = nc.dram_tensor("bb", [LAT], f32, kind="ExternalInput")
    mask = nc.dram_tensor("mask", [128, 896], f16, kind="ExternalInput")
    o = nc.dram_tensor("o", [T, D], f16, kind="ExternalOutput")

    with tile.TileContext(nc) as tc:
        with (
            tc.tile_pool(name="consts", bufs=1) as consts,
            tc.tile_pool(name="persist", bufs=1) as persist,
            tc.tile_pool(name="lpp", bufs=1) as lpp,
            tc.tile_pool(name="ccd", bufs=1, space="DRAM") as ccd,
        ):
            # mask[jj, c] = 1 iff c >= jj + 384, so cols [512, 640) are all
            # ones for every partition — doubles as the ones matrix for the
            # softmax-denominator matmul.
            mask_sb = consts.tile([128, 896], f16)
            g_sb = consts.tile([128, NLC], f32)
            b_sb = consts.tile([128, NLC], f32)
            eps_sb = consts.tile([128, 1], f32)
            nc.vector.memset(eps_sb, LN_EPS)
            ones_f32 = consts.tile([128, 128], f32)
            nc.vector.memset(ones_f32, 1.0)
            ones_sb = mask_sb[:, 512:640]

            qT_sb = persist.tile([128, HPC, T], f16)  # q transposed, per head
            latn_sb = persist.tile([128, NLC, T], f16)  # new latent (gathered)
            wk_sb = persist.tile([128, NLC, LAT], f16)
            wv_sb = persist.tile([128, NLC, LAT], f16)
            lp_sb = lpp.tile([128, NLC, PAST], f16)  # past latent
            # k/v weights ride the gpsimd DMA queue, in parallel with the
            # sync-queue weight/x streams of phase A
            nc.gpsimd.dma_start(
                wk_sb[:], wk[:, :].rearrange("(lc p) n -> p lc n", p=128)
            )
            nc.gpsimd.dma_start(
                wv_sb[:], wv[:, :].rearrange("(lc p) n -> p lc n", p=128)
            )
            cc_in = ccd.tile([LAT, 512], f16)
            cc_out = ccd.tile([4 * LAT, 512], f16)

            # ---- Phase A: latent down-projection + LayerNorm (first, so the
            # LN tail overlaps the q matmuls), then q projection.
            with (
                tc.tile_pool(name="wA", bufs=1) as wA,
                tc.tile_pool(name="stats", bufs=1) as stats,
                tc.tile_pool(name="psA", bufs=1, space="PSUM") as psA,
            ):
                wq_sb = wA.tile([128, NDC, LAT], f16)
                wd_sb = wA.tile([128, NDC, LAT], f16)
                # Weights ride the scalar (ACT) DMA queue, x the sync queue.

                latqb_sb = wA.tile([128, NLC, 512], f16)

                def ln_emit_stats():
                    # LN stats via ones-matmuls into PSUM (the qp2 bank pair)
                    # — keeps gpsimd out of phase A so the pool-close barrier
                    # cannot queue behind the collective on the gpsimd queue.
                    spair = psA.tile([128, 1024], f32, tag="qp3", name="sstat")
                    ssum = spair[:, 0:512]
                    ssq = spair[:, 512:1024]
                    for lc in range(NLC):
                        nc.tensor.matmul(
                            ssum,
                            lhsT=_r(ones_f32),
                            rhs=latq_sb[:, lc, :],
                            start=(lc == 0),
                            stop=(lc == NLC - 1),
                        )
                    for lc in range(NLC):
                        nc.tensor.matmul(
                            ssq,
                            lhsT=_r(ones_f32),
                            rhs=sqs[lc],
                            start=(lc == 0),
                            stop=(lc == NLC - 1),
                        )
                    mu = stats.tile([128, 512], f32, tag="mu", name="mu")
                    nc.vector.tensor_scalar_mul(mu, ssum, 1.0 / LAT)
                    vtmp = stats.tile([128, 512], f32, tag="vtmp", name="vtmp")
                    nc.vector.tensor_mul(vtmp, mu, mu)
                    sd = stats.tile([128, 512], f32, tag="sd", name="sd")
                    nc.vector.scalar_tensor_tensor(
                        out=sd,
                        in0=ssq,
                        scalar=1.0 / LAT,
                        in1=vtmp,
                        op0=OP.mult,
                        op1=OP.subtract,
                    )
                    # normalize tail, staged into the q-pass stream
                    box = {"mu": mu, "sd": sd}
                    ops = []
                    ops.append(lambda: nc.scalar.activation(
                        box["sd"], box["sd"], AF.Sqrt, bias=eps_sb))

                    def mkrstd():
                        box["rstd"] = stats.tile(
                            [128, 512], f32, tag="rstd", name="rstd")
                        nc.vector.reciprocal_approx_fast(box["rstd"], box["sd"])

                    ops.append(mkrstd)
                    for lc in range(NLC):
                        def mk1(lc=lc):
                            box["t1"] = stats.tile(
                                [128, 512], f32, tag="sq0", name="t1")
                            nc.vector.tensor_sub(
                                box["t1"], latq_sb[:, lc, :], box["mu"])

                        def mk2():
                            box["t2"] = stats.tile(
                                [128, 512], f32, tag="sq1", name="t2")
                            nc.vector.tensor_mul(
                                box["t2"], box["t1"], box["rstd"])

                        ops.append(mk1)
                        ops.append(mk2)
                        ops.append(lambda lc=lc: nc.vector.tensor_scalar(
                            latqb_sb[:, lc, :], box["t2"], g_sb[:, lc : lc + 1],
                            b_sb[:, lc : lc + 1], OP.mult, OP.add))
                    return ops

                pend_ln = []

                def drain_ln(k):
                    for _ in range(min(k, len(pend_ln))):
                        pend_ln.pop(0)()

                # down-projection for this core's T/4 token quarter
                # (64 matmuls).  x and its quarter are streamed once and
                # stay RESIDENT in SBUF (10MB fp16): the q projection then
                # has zero DMA dependency, so neither the x stream nor the
                # AllGather's DRAM round-trip can stall the PE.  PSUM is
                # managed as four [128,1024] bank pairs so psum->SBUF
                # copies run 1024 wide.
                latq_sb = wA.tile([128, NLC, 512], f32r)
                x_sb = wA.tile([128, NDC, T], f16)
                xq_sb = wA.tile([128, NDC, 512], f16)
                dpair = [
                    psA.tile([128, 1024], f32, tag=f"qp{i}", name=f"d_ps{i}")
                    for i in range(2)
                ]
                d_ps = [dpair[lc // 2][:, (lc % 2) * 512 : (lc % 2 + 1) * 512]
                        for lc in range(NLC)]
                # sync queue: x-quarter halves first (down-critical; each a
                # single full-rate 1MB DMA), consts, then the full x rows
                # in groups of 4 rows (4KB descriptors, full rate)
                xq_src = xq[:, :].rearrange("p (c t) -> p c t", c=NDC)
                nc.sync.dma_start(out=xq_sb[:, 0:8, :], in_=xq_src[:, 0:8, :])
                nc.sync.dma_start(out=xq_sb[:, 8:16, :], in_=xq_src[:, 8:16, :])
                nc.sync.dma_start(out=mask_sb, in_=mask[:, :])
                nc.sync.dma_start(
                    out=g_sb, in_=g[:].rearrange("(lc p) -> p lc", p=128)
                )
                nc.sync.dma_start(
                    out=b_sb, in_=b[:].rearrange("(lc p) -> p lc", p=128)
                )
                for lo in range(0, NDC, 4):
                    nc.sync.dma_start(
                        out=x_sb[:, lo : lo + 4, :],
                        in_=xT[lo * 128 : (lo + 4) * 128, :].rearrange(
                            "(c p) n -> p c n", p=128
                        ),
                    )
                # scalar queue: wd halves then wq halves — partition-major
                # single full-rate DMAs.  (The past-latent load is deferred
                # past the critical window; see below.)
                wd_src = wd[:, :].rearrange("p (c n) -> p c n", c=NDC)
                wq_src = wq[:, :].rearrange("p (c n) -> p c n", c=NDC)
                nc.scalar.dma_start(out=wd_sb[:, 0:8, :], in_=wd_src[:, 0:8, :])
                nc.scalar.dma_start(out=wd_sb[:, 8:16, :], in_=wd_src[:, 8:16, :])
                nc.scalar.dma_start(out=wq_sb[:, 0:8, :], in_=wq_src[:, 0:8, :])
                nc.scalar.dma_start(out=wq_sb[:, 8:16, :], in_=wq_src[:, 8:16, :])
                lp_src = lpT[:, :].rearrange("p (lc t) -> p lc t", lc=NLC)
                nc.scalar.dma_start(
                    out=lp_sb[:, :, 0:1024], in_=lp_src[:, :, 0:1024]
                )
                for dc in range(NDC):
                    for lc in range(NLC):
                        nc.tensor.matmul(
                            d_ps[lc],
                            lhsT=wd_sb[:, dc, lc * 128 : (lc + 1) * 128],
                            rhs=xq_sb[:, dc, :],
                            start=(dc == 0),
                            stop=(dc == NDC - 1),
                        )
                for pi in range(2):
                    dst = latq_sb[:, 2 * pi : 2 * pi + 2, :]
                    src = dpair[pi][:].rearrange("p (a b) -> p a b", a=2)
                    if pi == 0:
                        nc.vector.tensor_copy(dst, src)
                    else:
                        nc.scalar.copy(out=dst, in_=src)
                # squares, LN stats and the normalize chain immediately:
                # latqb must be in DRAM early so the AllGather can trigger
                # at tp0-end (with x resident there is no DMA stream left
                # for its D2D traffic to disturb)
                sqs = []
                for lc in range(NLC):
                    sq = stats.tile([128, 512], f32r, tag=f"sq{lc}", name="sq")
                    nc.scalar.square(sq, latq_sb[:, lc, :])
                    sqs.append(sq)
                pend_ln.extend(ln_emit_stats())

                # q projection from resident x.  Pair pi = half*2 + qc//2.
                def q_mm(pr, pi, dc, tp):
                    half, qg = pi // 2, pi % 2
                    for sub in range(2):
                        qc = qg * 2 + sub
                        nc.tensor.matmul(
                            pr[:, sub * 512 : (sub + 1) * 512],
                            lhsT=wq_sb[:, dc, qc * 128 : (qc + 1) * 128],
                            rhs=x_sb[
                                :, dc,
                                tp * 1024 + half * 512 : tp * 1024
                                + (half + 1) * 512,
                            ],
                            start=(dc == 0),
                            stop=(dc == NDC - 1),
                        )

                def q_copy(pr, pi, tp, eng):
                    half, qg = pi // 2, pi % 2
                    tt = tp * 2 + half
                    dst = qT_sb[:, qg * 2 : qg * 2 + 2, tt * 512 : (tt + 1) * 512]
                    src = pr[:].rearrange("p (a b) -> p a b", a=2)
                    if eng == 0:
                        nc.scalar.copy(out=dst, in_=src)
                    else:
                        nc.vector.tensor_copy(dst, src)

                # tp0: dc-major, qp2 first within each dc (qp0/qp1 wait for
                # the latq copies, qp3 for the stats readers — both clear
                # within the first few matmuls).  The LN normalize chain
                # drains into this stream.
                q0 = {
                    pi: psA.tile([128, 1024], f32, tag=f"qp{pi}", name=f"q0_{pi}")
                    for pi in (2, 0, 1, 3)
                }
                for dc in range(NDC):
                    drain_ln(4)
                    for pi in (2, 0, 1, 3):
                        q_mm(q0[pi], pi, dc, 0)
                for k, pi in enumerate((2, 3, 0, 1)):
                    q_copy(q0[pi], pi, 0, k % 2)
                drain_ln(999)
                # ship this core's normalized latent quarter and fire the
                # gather NOW: its DRAM round-trip overlaps tp1 (which reads
                # only resident SBUF) and B-past, and latn lands with wide
                # margin before B-new
                nc.sync.dma_start(
                    out=cc_in[:].rearrange("(lc p) t -> p lc t", p=128),
                    in_=latqb_sb[:],
                )
                nc.gpsimd.collective_compute(
                    "AllGather",
                    mybir.AluOpType.bypass,
                    replica_groups=[[0, 1, 2, 3], [4, 5, 6, 7]],
                    ins=[cc_in.opt()],
                    outs=[cc_out.opt()],
                )
                for rk in range(4):
                    nc.gpsimd.dma_start(
                        latn_sb[:, :, rk * 512 : (rk + 1) * 512],
                        cc_out[rk * 512 : (rk + 1) * 512, :].rearrange(
                            "(lc p) t -> p lc t", p=128
                        ),
                    )

                # tp1: bank-pair-major from resident x — pure PE streaming
                def q_block(pi):
                    pr = psA.tile(
                        [128, 1024], f32, tag=f"qp{pi}", name=f"q1_{pi}"
                    )
                    for dc in range(NDC):
                        q_mm(pr, pi, dc, 1)
                    return pr

                q1 = {}
                for pi in (2, 0, 1, 3):
                    q1[pi] = q_block(pi)
                for k, pi in enumerate((0, 1, 2, 3)):
                    q_copy(q1[pi], pi, 1, k % 2)

            # ---- Phase B: k/v up-projection for all 4 heads
            with tc.tile_pool(name="kvbuf", bufs=1) as kvp:
                kT_sb = kvp.tile([128, HPC, S], f16)
                v_sb = kvp.tile([128, NJB, LAT], f16)
                ao_sb = kvp.tile([128, HPC, T], f16)  # attn out transposed
                with tc.tile_pool(name="psB", bufs=1, space="PSUM") as psB:

                    def latf(lc, g_):
                        if g_ < PAST // 512:
                            return lp_sb[:, lc, g_ * 512 : (g_ + 1) * 512]
                        gg = g_ - PAST // 512
                        return latn_sb[:, lc, gg * 512 : (gg + 1) * 512]

                    for g_ in range(S // 512):
                        ssl = slice(g_ * 512, (g_ + 1) * 512)
                        if g_ == 0:
                            # prefetch the second past-latent half (scalar
                            # queue; gpsimd is owned by the collective)
                            lps = lpT[:, :].rearrange(
                                "p (lc t) -> p lc t", lc=NLC
                            )
                            nc.scalar.dma_start(
                                out=lp_sb[:, :, 1024:2048],
                                in_=lps[:, :, 1024:2048],
                            )
                        if g_ == 5:
                            # pre-load the ACT Exp table (1.28us) here, where
                            # the scalar engine has slack, instead of at the
                            # first attention EXP
                            warm = consts.tile([128, 1], f16, name="warm")
                            nc.scalar.activation(warm, eps_sb, AF.Exp)
                        for h in range(HPC):
                            hsl = slice(h * 128, (h + 1) * 128)
                            k_ps = psB.tile(
                                [128, 512], f32, tag="kps", bufs=3, name="k_ps"
                            )
                            for lc in range(NLC):
                                nc.tensor.matmul(
                                    k_ps,
                                    lhsT=wk_sb[:, lc, hsl],
                                    rhs=latf(lc, g_),
                                    start=(lc == 0),
                                    stop=(lc == NLC - 1),
                                )
                            if h % 2 == 0:
                                nc.vector.tensor_copy(kT_sb[:, h, ssl], k_ps)
                            else:
                                nc.scalar.copy(out=kT_sb[:, h, ssl], in_=k_ps)
                        for j4 in range(4):
                            v_ps = psB.tile(
                                [128, 512], f32, tag="vps", bufs=3, name="v_ps"
                            )
                            for lc in range(NLC):
                                nc.tensor.matmul(
                                    v_ps,
                                    lhsT=latf(lc, g_)[
                                        :, j4 * 128 : (j4 + 1) * 128
                                    ],
                                    rhs=wv_sb[:, lc, :],
                                    start=(lc == 0),
                                    stop=(lc == NLC - 1),
                                )
                            if j4 % 2 == 0:
                                nc.scalar.copy(out=v_sb[:, g_ * 4 + j4, :], in_=v_ps)
                            else:
                                nc.vector.tensor_copy(v_sb[:, g_ * 4 + j4, :], v_ps)

                # ---- Phase C: attention per head / query tile.  Scores for
                # two adjacent key blocks share one [128,1024] psum pair and
                # one EXP instruction.
                with (
                    tc.tile_pool(name="pp", bufs=8) as pp,
                    tc.tile_pool(name="pdp", bufs=2) as pdp,
                    tc.tile_pool(name="ctmp", bufs=2) as ctmp,
                    tc.tile_pool(name="psC", bufs=1, space="PSUM") as psC,
                ):
                  wo_sb = kvp.tile([128, HPC, D], f16)
                  # wo rides the gpsimd queue behind the latn scatter: it
                  # lands mid-B, off the sync queue's congested A->B window
                  nc.gpsimd.dma_start(
                      wo_sb[:],
                      wo[:, :].rearrange("(hc p) n -> p hc n", p=128),
                  )
                  for h in range(HPC):
                    hsl = slice(h * 128, (h + 1) * 128)
                    for tt in range(NTT):
                        tsl = slice(tt * 512, (tt + 1) * 512)
                        nvis = PAST // 128 + 4 * (tt + 1)  # 20/24/28/32
                        npair = nvis // 2
                        # attention accumulator and softmax denominator share
                        # one psum tile (same lifetime)
                        ad = psC.tile([128, 1024], f32, tag="ad", bufs=2, name="ad")
                        attn_ps = ad[:, 0:512]
                        den_ps = ad[:, 512:1024]
                        grp = []
                        e = None
                        den_started = False
                        for jp in range(npair):
                            jb0, jb1 = 2 * jp, 2 * jp + 1
                            sp = psC.tile(
                                [128, 1024], f32, tag="sp", bufs=2, name="s_ps"
                            )
                            nc.tensor.matmul(
                                sp[:, 0:512],
                                lhsT=kT_sb[:, h, jb0 * 128 : (jb0 + 1) * 128],
                                rhs=qT_sb[:, h, tsl],
                                start=True,
                                stop=True,
                            )
                            nc.tensor.matmul(
                                sp[:, 512:1024],
                                lhsT=kT_sb[:, h, jb1 * 128 : (jb1 + 1) * 128],
                                rhs=qT_sb[:, h, tsl],
                                start=True,
                                stop=True,
                            )
                            pr = pp.tile([128, 1024], f16, tag="p", name="p")
                            nc.scalar.activation(pr, sp, AF.Exp, scale=SCALE)
                            for half, jb in ((0, jb0), (1, jb1)):
                                jbn = jb - PAST // 128
                                if jbn >= 0 and jbn // 4 == tt:
                                    rr = (jbn % 4) * 128
                                    hslc = slice(half * 512, (half + 1) * 512)
                                    nc.vector.tensor_mul(
                                        pr[:, hslc],
                                        pr[:, hslc],
                                        mask_sb[:, 384 - rr : 896 - rr],
                                    )
                            for half, jb in ((0, jb0), (1, jb1)):
                                # diagonal blocks: only queries >= rr are
                                # visible (the mask zeroed the rest), so the
                                # PV matmul streams just the visible columns
                                jbn = jb - PAST // 128
                                rr = 0
                                if jbn >= 0 and jbn // 4 == tt:
                                    rr = (jbn % 4) * 128
                                nc.tensor.matmul(
                                    attn_ps[:, rr:512],
                                    lhsT=v_sb[:, jb, hsl],
                                    rhs=pr[:, half * 512 + rr : (half + 1) * 512],
                                    start=(jp == 0 and half == 0),
                                    stop=(jp == npair - 1 and half == 1),
                                    skip_group_check=True,
                                )
                            # denominator: accumulate exp'd pairs 1024-wide
                            # on DVE, fold + ones-matmul per <=8 pairs
                            grp.append(pr)
                            if len(grp) == 2:
                                e = pdp.tile(
                                    [128, 1024], f16, tag="pd", bufs=2, name="e"
                                )
                                nc.vector.tensor_add(e, grp[0], grp[1])
                            elif len(grp) > 2:
                                nc.vector.tensor_add(e, e, pr)
                            if len(grp) == 8 or jp == npair - 1:
                                ef = pdp.tile(
                                    [128, 512], f16, tag="pf", bufs=2, name="ef"
                                )
                                nc.vector.tensor_add(
                                    ef, e[:, 0:512], e[:, 512:1024]
                                )
                                nc.tensor.matmul(
                                    den_ps,
                                    lhsT=ones_sb,
                                    rhs=ef,
                                    start=not den_started,
                                    stop=(jp == npair - 1),
                                )
                                den_started = True
                                grp = []
                        rec = ctmp.tile([128, 512], f32, tag="rec", name="rec")
                        nc.vector.reciprocal_approx_fast(rec, den_ps)
                        nc.vector.tensor_mul(ao_sb[:, h, tsl], attn_ps, rec)

                # ---- Phase D: output projection (fp16), 1024-wide copies
                # and DMA descriptors
                with (
                    tc.tile_pool(name="ost", bufs=4) as ost,
                    tc.tile_pool(name="psD", bufs=1, space="PSUM") as psD,
                ):
                    for dt2 in range(D // 1024):
                        for tc_ in range(T // 128):
                            o_ps = psD.tile(
                                [128, 1024], f32, tag="ops", bufs=4, name="o_ps"
                            )
                            for half in range(2):
                                csl = slice(
                                    (dt2 * 2 + half) * 512,
                                    (dt2 * 2 + half + 1) * 512,
                                )
                                for hc in range(HPC):
                                    nc.tensor.matmul(
                                        o_ps[:, half * 512 : (half + 1) * 512],
                                        lhsT=ao_sb[
                                            :, hc, tc_ * 128 : (tc_ + 1) * 128
                                        ],
                                        rhs=wo_sb[:, hc, csl],
                                        start=(hc == 0),
                                        stop=(hc == HPC - 1),
                                    )
                            o_sb = ost.tile(
                                [128, 1024], f16, tag="osb", name="o_sb"
                            )
                            if tc_ % 2 == 0:
                                nc.scalar.copy(out=o_sb, in_=o_ps)
                            else:
                                nc.vector.tensor_copy(o_sb, o_ps)
                            nc.sync.dma_start(
                                out=o[
                                    tc_ * 128 : (tc_ + 1) * 128,
                                    dt2 * 1024 : (dt2 + 1) * 1024,
                                ],
                                in_=o_sb,
                            )

    nc.compile()
    return nc


def _get_nc():
    if "nc" not in _CACHE:
        _CACHE["nc"] = _build()
    return _CACHE["nc"]


def _make_mask():
    # B[jj, c] = 1.0 iff c >= jj + 384; sliced at 384-r it gives the
    # causal staircase "visible iff i >= jj + r" for r in {0,128,256,384}.
    jj = np.arange(128)[:, None]
    cc = np.arange(896)[None, :]
    return (cc >= jj + 384)


def _swz(a):
    # [C*128, N] -> partition-major [128, C*N]: row p holds chunks
    # (0..C-1) of the original rows {c*128+p}, each N wide and contiguous
    c = a.shape[0] // 128
    return np.ascontiguousarray(
        a.reshape(c, 128, a.shape[1]).transpose(1, 0, 2).reshape(128, -1)
    )


def _in_maps(x, latent_prev, Wq, Wdown, Wk_up, Wv_up, ln_g, ln_b, Wo):
    f = lambda a: np.ascontiguousarray(np.asarray(a, dtype=np.float32))
    h = lambda a: np.ascontiguousarray(
        np.asarray(a, dtype=np.float32).astype(np.float16)
    )
    mask = _make_mask().astype(np.float16)
    wd_h = _swz(h(Wdown))
    maps = []
    for bi in range(2):
        xTb = h(np.asarray(x)[bi].T)
        lpTb = _swz(h(np.asarray(latent_prev)[bi].T))
        for hg in range(4):
            sl = slice(hg * 512, (hg + 1) * 512)
            maps.append(
                {
                    "xT": xTb,
                    "xq": _swz(np.ascontiguousarray(xTb[:, sl])),
                    "lpT": lpTb,
                    "wq": _swz(h(np.asarray(Wq)[:, sl])),
                    "wd": wd_h,
                    "wk": h(np.asarray(Wk_up)[:, sl]),
                    "wv": h(np.asarray(Wv_up)[:, sl]),
                    "wo": h(np.asarray(Wo)[sl, :]),
                    "g": f(ln_g),
                    "bb": f(ln_b),
                    "mask": mask,
                }
            )
    return maps


def run(trace=False, **inputs):
    from concourse.bass_utils import run_bass_kernel_spmd

    nc = _get_nc()
    maps = _in_maps(**inputs)
    res = run_bass_kernel_spmd(nc, maps, core_ids=list(range(8)), trace=trace)
    outs = [
        np.asarray(res.results[c]["o"], dtype=np.float32) for c in range(8)
    ]
    out = np.stack(
        [
            outs[0] + outs[1] + outs[2] + outs[3],
            outs[4] + outs[5] + outs[6] + outs[7],
        ],
        axis=0,
    ).astype(np.float32)
    return out, res


def kernel(**inputs):
    out, _ = run(trace=False, **inputs)
    return out
